# revision 1
# baseline (speedup 1.0000x reference)
"""GroupDense kernel for Trainium2 (8 NeuronCores, SPMD data-parallel over batch).

y[b,s,g*64+v] = relu(sum_u x[b,s,g*64+u] * w[g,u,v])
x: [8, 2048, 4096] fp32, w: [64, 64, 64] fp32.

Per-core: core i processes batch i ([2048, 4096], 32 MB in / 32 MB out).
Weights are packed host-side into 32 block-diagonal [128,128] tiles (two
64x64 groups per tile) so each matmul contracts over K=128 with full PE
utilization. x must be transposed on-chip (contraction dim on partitions):
PE transpose -> PSUM -> DVE copy -> SBUF, then fp32 matmul, ReLU on ACT.
"""

import numpy as np

import concourse.bass as bass
import concourse.mybir as mybir
import concourse.tile as tile
from concourse import bacc
from concourse.bass import ds, ts
from concourse.bass_utils import run_bass_kernel_spmd
from concourse.masks import make_identity

B, S, C = 8, 2048, 4096
U = 64
G = C // U  # 64 groups
NCORES = 8
TOK = (B * S) // NCORES  # 2048 tokens per core
P = 128
CB = C // P   # 32 channel blocks (2 groups each)
TT = TOK // P  # 16 token tiles
QUADS = CB // 4  # 8 quads of 4 channel blocks

F32 = mybir.dt.float32

_cached_nc = None
_cfg = {}


def _build():
    global _cached_nc
    if _cached_nc is not None:
        return _cached_nc

    nc = bacc.Bacc("TRN2", target_bir_lowering=False)

    x_d = nc.dram_tensor("x", [TOK, C], F32, kind="ExternalInput")
    # host pre-packs weights partition-major so the DMA is contiguous.
    # split_mm: [P, (G//2)*U] (rows 0-63 even groups, 64-127 odd groups);
    # else: [P, CB*P] block-diagonal pairs.
    WFREE = (G // 2) * U if _cfg.get("split_mm", False) else CB * P
    w_d = nc.dram_tensor("w2", [P, WFREE], F32, kind="ExternalInput")
    y_d = nc.dram_tensor("y", [TOK, C], F32, kind="ExternalOutput")

    F32R = mybir.dt.float32r
    OCT = 8  # channel blocks per unit: 2 PSUM banks per psum tile
    UNITS = CB // OCT

    with tile.TileContext(nc) as tc:
        with (
            tc.tile_pool(name="consts", bufs=1) as consts,
            tc.tile_pool(name="wpool", bufs=1) as wpool,
            tc.tile_pool(name="xpool", bufs=3) as xpool,
            tc.tile_pool(name="xtpool", bufs=3) as xtpool,
            tc.tile_pool(name="ypool", bufs=3) as ypool,
            tc.tile_pool(name="psT", bufs=2, space="PSUM") as psT,
            tc.tile_pool(name="psY", bufs=2, space="PSUM") as psY,
        ):
            ident = consts.tile([P, P], F32)
            make_identity(nc, ident[:])

            # weights first: the first matmul needs them; rides ACT's HWDGE
            # queue so it runs concurrently with the first x chunk on Sync's
            if _cfg.get("split_mm", False):
                w_s = wpool.tile([P, G // 2, U], F32)
                w_sbd = None
            else:
                w_sbd = wpool.tile([P, CB, P], F32)
                w_s = w_sbd
            nc.scalar.dma_start(w_s[:], w_d[:, :])

            CHUNK = OCT * P  # 1024 channels per unit
            IN_CHUNKS = _cfg.get("in_chunks", 2)
            OUT_EVERY = _cfg.get("out_every", 2)  # store y every N units
            for tt in range(TT):
                x_t = xpool.tile([P, C], F32)
                icw = C // IN_CHUNKS
                for ic in range(IN_CHUNKS):
                    nc.sync.dma_start(
                        x_t[:, ds(ic * icw, icw)],
                        x_d[ts(tt, P), ds(ic * icw, icw)],
                    )
                y_t = ypool.tile([P, C], F32)

                for q in range(UNITS):
                    pT = psT.tile([P, OCT, P], F32)
                    for j in range(OCT):
                        cb = OCT * q + j
                        nc.tensor.transpose(
                            pT[:, j, :], x_t[:, ts(cb, P)], ident[:]
                        )
                    xT = xtpool.tile([P, OCT, P], F32)
                    nc.vector.tensor_copy(xT[:], pT[:])

                    if _cfg.get("split_mm", False):
                        # two K=64 group-matmuls on distinct PE row-groups
                        # (0 and 64) issued back-to-back run concurrently;
                        # they MUST land in separate PSUM banks (same-bank
                        # disjoint-column pairs fault the exec unit).
                        pA = psY.tile([P, OCT, U], F32)
                        pB = psY.tile([P, OCT, U], F32)
                        for j in range(OCT):
                            cb = OCT * q + j
                            nc.tensor.matmul(
                                pA[:, j, :], xT[0:U, j, :], w_s[0:U, cb, :],
                                start=True, stop=True,
                            )
                            nc.tensor.matmul(
                                pB[:, j, :], xT[U:P, j, :], w_s[U:P, cb, :],
                                start=True, stop=True,
                            )
                        ys = y_t[:, ds(q * CHUNK, CHUNK)].rearrange(
                            "p (j u2) -> p j u2", u2=P
                        )
                        nc.scalar.activation(
                            ys[:, :, 0:U], pA[:],
                            mybir.ActivationFunctionType.Relu,
                        )
                        nc.scalar.activation(
                            ys[:, :, U:P], pB[:],
                            mybir.ActivationFunctionType.Relu,
                        )
                    else:
                        pY = psY.tile([P, OCT, P], F32)
                        for j in range(OCT):
                            cb = OCT * q + j
                            nc.tensor.matmul(
                                pY[:, j, :], xT[:, j, :], w_sbd[:, cb, :],
                                start=True, stop=True,
                            )
                        nc.scalar.activation(
                            y_t[:, ds(q * CHUNK, CHUNK)], pY[:],
                            mybir.ActivationFunctionType.Relu,
                        )
                    if (q + 1) % OUT_EVERY == 0:
                        ow = OUT_EVERY * CHUNK
                        oc = (q + 1) // OUT_EVERY - 1
                        nc.sync.dma_start(
                            y_d[ts(tt, P), ds(oc * ow, ow)],
                            y_t[:, ds(oc * ow, ow)],
                        )

    nc.compile()
    _cached_nc = nc
    return nc


def _pack_weights(kern):
    if _cfg.get("split_mm", False):
        # [P, (G//2)*U]: rows 0-63 = even groups' [u, v] blocks along the
        # free dim, rows 64-127 = odd groups'. Matches SBUF tile [P, G//2, U].
        top = kern[0::2].transpose(1, 0, 2).reshape(U, (G // 2) * U)
        bot = kern[1::2].transpose(1, 0, 2).reshape(U, (G // 2) * U)
        return np.ascontiguousarray(np.concatenate([top, bot], axis=0))
    w2 = np.zeros((CB, P, P), dtype=np.float32)
    w2[:, :U, :U] = kern[0::2]
    w2[:, U:, U:] = kern[1::2]
    return np.ascontiguousarray(w2.transpose(1, 0, 2).reshape(P, CB * P))


def kernel(x, kernel):
    x = np.ascontiguousarray(x, dtype=np.float32)
    w2 = _pack_weights(np.asarray(kernel, dtype=np.float32))

    nc = _build()
    in_maps = [
        {"x": np.ascontiguousarray(x[i].reshape(TOK, C)), "w2": w2}
        for i in range(NCORES)
    ]
    res = run_bass_kernel_spmd(nc, in_maps, list(range(NCORES)))
    y = np.stack([res.results[i]["y"] for i in range(NCORES)], axis=0)
    return y.reshape(B, S, C)



# revision 2
# speedup vs baseline: 2.4063x; 2.4063x over previous
"""GroupDense kernel for Trainium2 (8 NeuronCores, SPMD data-parallel over batch).

y[b,s,g*64+v] = relu(sum_u x[b,s,g*64+u] * w[g,u,v])
x: [8, 2048, 4096] fp32, w: [64, 64, 64] fp32.

Per-core: core i processes batch i. Host pre-transposes/casts the shard to
x^T [C, TOK] bf16 so the contraction dim lands on SBUF partitions with no
on-chip transpose, and packs weights into 32 block-diagonal [128,128] bf16
tiles (two 64x64 groups each). The matmul runs weight-stationary
(lhsT = w block, rhs = x^T streaming 512 tokens) so the output is y^T
[outch, tok]; ReLU (split across ACT and DVE) writes bf16, stores go out on
the ACT HWDGE ring while loads ride the SP ring. Host un-transposes y^T and
upcasts to fp32. HBM traffic is 16 MB in + 16 MB out per core.
"""

import numpy as np
import ml_dtypes

import concourse.bass as bass
import concourse.mybir as mybir
import concourse.tile as tile
from concourse import bacc
from concourse.bass import ds, ts
from concourse.bass_utils import run_bass_kernel_spmd

B, S, C = 8, 2048, 4096
U = 64
G = C // U  # 64 groups
NCORES = 8
TOK = (B * S) // NCORES  # 2048 tokens per core
P = 128
CB = C // P  # 32 channel blocks (2 groups each)
NSEG = TOK // 512  # 4 matmul segments of 512 tokens per stripe

F32 = mybir.dt.float32
BF16 = mybir.dt.bfloat16
BF16NP = ml_dtypes.bfloat16

_cached_nc = None


def _build():
    global _cached_nc
    if _cached_nc is not None:
        return _cached_nc

    nc = bacc.Bacc("TRN2", target_bir_lowering=False)

    x_d = nc.dram_tensor("x", [C, TOK], BF16, kind="ExternalInput")
    w_d = nc.dram_tensor("w2", [P, CB * P], BF16, kind="ExternalInput")
    y_d = nc.dram_tensor("y", [C, TOK], BF16, kind="ExternalOutput")

    with tile.TileContext(nc) as tc:
        with (
            tc.tile_pool(name="wpool", bufs=1) as wpool,
            tc.tile_pool(name="xpool", bufs=4) as xpool,
            tc.tile_pool(name="ypool", bufs=4) as ypool,
            tc.tile_pool(name="ps", bufs=2, space="PSUM") as ps,
        ):
            w_s = wpool.tile([P, CB, P], BF16)
            nc.scalar.dma_start(w_s[:], w_d[:, :])

            for cb in range(CB):
                x_t = xpool.tile([P, TOK], BF16)
                nc.sync.dma_start(x_t[:], x_d[ts(cb, P), :])

                y_t = ypool.tile([P, TOK], BF16)
                pY = ps.tile([P, NSEG, 512], F32)
                for i in range(NSEG):
                    nc.tensor.matmul(
                        pY[:, i, :],
                        w_s[:, cb, :],
                        x_t[:, ds(i * 512, 512)],
                        start=True,
                        stop=True,
                    )
                half = (NSEG // 2) * 512
                nc.scalar.activation(
                    y_t[:, 0:half],
                    pY[:, 0 : NSEG // 2, :],
                    mybir.ActivationFunctionType.Relu,
                )
                nc.vector.tensor_scalar_max(
                    y_t[:, half:TOK], pY[:, NSEG // 2 : NSEG, :], 0.0
                )
                nc.scalar.dma_start(y_d[ts(cb, P), :], y_t[:])

    nc.compile()
    _cached_nc = nc
    return nc


def _pack_weights(kern):
    # [P, CB*P] bf16: block-diagonal pairs, partition-major (u within block
    # on partitions; blocks x out-channel along the free dim).
    w2 = np.zeros((CB, P, P), dtype=np.float32)
    w2[:, :U, :U] = kern[0::2]
    w2[:, U:, U:] = kern[1::2]
    return np.ascontiguousarray(
        w2.transpose(1, 0, 2).reshape(P, CB * P).astype(BF16NP)
    )


def prep_inputs(x, kern):
    x = np.asarray(x, dtype=np.float32)
    w2 = _pack_weights(np.asarray(kern, dtype=np.float32))
    in_maps = [
        {
            "x": np.ascontiguousarray(x[i].reshape(TOK, C).T.astype(BF16NP)),
            "w2": w2,
        }
        for i in range(NCORES)
    ]
    return in_maps


def postprocess(res):
    y = np.stack(
        [
            np.asarray(res.results[i]["y"]).astype(np.float32).T
            for i in range(NCORES)
        ],
        axis=0,
    )
    return np.ascontiguousarray(y.reshape(B, S, C))


def kernel(x, kernel):
    nc = _build()
    in_maps = prep_inputs(x, kernel)
    res = run_bass_kernel_spmd(nc, in_maps, list(range(NCORES)))
    return postprocess(res)


# revision 3
# speedup vs baseline: 2.4839x; 1.0323x over previous
"""GroupDense kernel for Trainium2 (8 NeuronCores, SPMD data-parallel over batch).

y[b,s,g*64+v] = relu(sum_u x[b,s,g*64+u] * w[g,u,v])
x: [8, 2048, 4096] fp32, w: [64, 64, 64] fp32.

Per-core: core i processes batch i. Host pre-transposes/casts the shard to
x^T [C, TOK] bf16 so the contraction dim lands on SBUF partitions with no
on-chip transpose, and packs weights into 32 block-diagonal [128,128] bf16
tiles (two 64x64 groups each). The matmul runs weight-stationary
(lhsT = w block, rhs = x^T streaming 512 tokens) so the output is y^T
[outch, tok]; ReLU (split across ACT and DVE) writes bf16, stores go out on
the ACT HWDGE ring while loads ride the SP ring. Host un-transposes y^T and
upcasts to fp32. HBM traffic is 16 MB in + 16 MB out per core.
"""

import numpy as np
import ml_dtypes

import concourse.bass as bass
import concourse.mybir as mybir
import concourse.tile as tile
from concourse import bacc
from concourse.bass import ds, ts
from concourse.bass_utils import run_bass_kernel_spmd

B, S, C = 8, 2048, 4096
U = 64
G = C // U  # 64 groups
NCORES = 8
TOK = (B * S) // NCORES  # 2048 tokens per core
P = 128
CB = C // P  # 32 channel blocks (2 groups each)
NSEG = TOK // 512  # 4 matmul segments of 512 tokens per stripe

F32 = mybir.dt.float32
BF16 = mybir.dt.bfloat16
BF16NP = ml_dtypes.bfloat16

_cached_nc = None


def _build():
    global _cached_nc
    if _cached_nc is not None:
        return _cached_nc

    nc = bacc.Bacc("TRN2", target_bir_lowering=False)

    x_d = nc.dram_tensor("x", [C, TOK], BF16, kind="ExternalInput")
    w_d = nc.dram_tensor("w2", [P, CB * P], BF16, kind="ExternalInput")
    y_d = nc.dram_tensor("y", [C, TOK], BF16, kind="ExternalOutput")

    with tile.TileContext(nc) as tc:
        with (
            tc.tile_pool(name="wpool", bufs=1) as wpool,
            tc.tile_pool(name="xpool", bufs=6) as xpool,
            tc.tile_pool(name="ypool", bufs=6) as ypool,
            tc.tile_pool(name="ps", bufs=2, space="PSUM") as ps,
        ):
            # w in 4 chunks so the first matmul (needs chunk 0 only) starts
            # ~6us earlier; that pulls the whole store stream left, keeping
            # both HWDGE rings streaming through the ramp.
            w_s = wpool.tile([P, CB, P], BF16)
            WBLK = CB // 4
            for c in range(4):
                nc.scalar.dma_start(
                    w_s[:, ds(c * WBLK, WBLK), :],
                    w_d[:, ds(c * WBLK * P, WBLK * P)],
                )

            for cb in range(CB):
                x_t = xpool.tile([P, TOK], BF16)
                nc.sync.dma_start(x_t[:], x_d[ts(cb, P), :])

                y_t = ypool.tile([P, TOK], BF16)
                pY = ps.tile([P, NSEG, 512], F32)
                for i in range(NSEG):
                    nc.tensor.matmul(
                        pY[:, i, :],
                        w_s[:, cb, :],
                        x_t[:, ds(i * 512, 512)],
                        start=True,
                        stop=True,
                    )
                half = (NSEG // 2) * 512
                nc.scalar.activation(
                    y_t[:, 0:half],
                    pY[:, 0 : NSEG // 2, :],
                    mybir.ActivationFunctionType.Relu,
                )
                nc.vector.tensor_scalar_max(
                    y_t[:, half:TOK], pY[:, NSEG // 2 : NSEG, :], 0.0
                )
                # tail: once loads are done the sync ring idles — drain the
                # last stores across both rings.
                store_eng = nc.sync if (cb >= CB - 6 and cb % 2 == 0) else nc.scalar
                store_eng.dma_start(y_d[ts(cb, P), :], y_t[:])

    nc.compile()
    _cached_nc = nc
    return nc


def _pack_weights(kern):
    # [P, CB*P] bf16: block-diagonal pairs, partition-major (u within block
    # on partitions; blocks x out-channel along the free dim).
    w2 = np.zeros((CB, P, P), dtype=np.float32)
    w2[:, :U, :U] = kern[0::2]
    w2[:, U:, U:] = kern[1::2]
    return np.ascontiguousarray(
        w2.transpose(1, 0, 2).reshape(P, CB * P).astype(BF16NP)
    )


def prep_inputs(x, kern):
    x = np.asarray(x, dtype=np.float32)
    w2 = _pack_weights(np.asarray(kern, dtype=np.float32))
    in_maps = [
        {
            "x": np.ascontiguousarray(x[i].reshape(TOK, C).T.astype(BF16NP)),
            "w2": w2,
        }
        for i in range(NCORES)
    ]
    return in_maps


def postprocess(res):
    y = np.stack(
        [
            np.asarray(res.results[i]["y"]).astype(np.float32).T
            for i in range(NCORES)
        ],
        axis=0,
    )
    return np.ascontiguousarray(y.reshape(B, S, C))


def kernel(x, kernel):
    nc = _build()
    in_maps = prep_inputs(x, kernel)
    res = run_bass_kernel_spmd(nc, in_maps, list(range(NCORES)))
    return postprocess(res)
